# revision 28
# baseline (speedup 1.0000x reference)
"""Bass/Tile Trainium2 kernel for nn_Attention_7284264534326.

Single-head attention, B=8, S=2048, D=1024:
    q = (x1 @ wq) * D**-0.5 ; k = x2 @ wk ; v = x2 @ wv
    a = softmax(q @ k^T + mask * -1e9, axis=-1)
    out = relu(a @ v) @ wo

Sharding: data-parallel over batch; one batch element per NeuronCore (8 cores).

Structural optimizations vs a dense implementation:
  - maskSeq==1 keys contribute exactly zero (exp(-1e9) == 0 in f32), so the
    host gathers only the unmasked rows of x2 (~1024 of 2048), padded to a
    fixed K_pad (multiple of 128).  k/v projections, scores and a@v shrink
    by ~44%.  Pad slots get k=v=0 plus an additive -1e9 exp-bias so they
    contribute exactly 0 to numerator and denominator.
  - scores are reassociated: (x1@wq)@(x2g@wk)^T == x1 @ (wq@wk^T) @ x2g^T.
    W2T = (wk @ wq^T * scale) costs 2*D^3 and G2 = W2 @ x2g^T costs
    2*D^2*K_pad, replacing the q and k projections (2*S*D^2 + 2*K_pad*D^2)
    -- a ~28us PE saving since S > K_pad >= weights.
  - x1/x2 transposed to [D, *] on the host; no PE transposes anywhere;
    every PE op is a productive matmul with moving-operand N >= 384.
  - softmax denominator via M=1 ones-column matmuls accumulating a [1, Q]
    PSUM row (1-column LDWEIGHTS is ~free), transposed to per-partition
    layout with 4 tiny K=1 matmuls per q-tile.

Per-core dataflow (all matmul operands bf16, PSUM accumulation f32):
  phase 1: W2T[d2,d1] = matmul(lhsT=wkT, rhs=wqT) * 1/32 on evac
           G2[d1,kp]  = matmul(lhsT=W2T, rhs=x2gT)
           V[kp,e]    = matmul(lhsT=x2gT, rhs=wv)
  phase 2 (per 512-query tile):
           scores^T[kp,q] = matmul(lhsT=G2, rhs=x1T); exp fused into ACT
           evacuation with pad bias; denom row via ones-matmuls;
           yU^T[e,q] = matmul(lhsT=V, rhs=exp^T); relu on evac;
           out[q,f] = matmul(lhsT=z^T, rhs=wo) scaled by 1/denom on evac.
"""

import numpy as np
from contextlib import ExitStack

B, S, D = 8, 2048, 1024
P = 128
DC = D // P       # 8 chunks of the depth/contraction dim
EC = D // P       # 8 chunks of the embedding dim
Q_TILE = 512      # queries per attention tile
NQT = S // Q_TILE # 4
QS = Q_TILE // P  # 4 query sub-chunks of 128
N_CORES = 8
QSCALE = float(D) ** -0.5  # 1/32

_cached_nc = {}


def _ksplits(k_pad):
    """Split k_pad into free-dim slices <=512, each a multiple of 128."""
    nsl = -(-k_pad // 512)
    base = (k_pad // nsl) // P * P
    widths = [base] * nsl
    rem = k_pad - base * nsl
    i = 0
    while rem > 0:
        widths[i] += P
        rem -= P
        i += 1
    out, s0 = [], 0
    for w in widths:
        out.append((s0, w))
        s0 += w
    return out


def _build(k_pad):
    import concourse.tile as tile
    from concourse import bacc, mybir

    f32 = mybir.dt.float32
    bf16 = mybir.dt.bfloat16
    AF = mybir.ActivationFunctionType
    KC = k_pad // P

    nc = bacc.Bacc("TRN2", target_bir_lowering=False, debug=False,
                   enable_asserts=False, num_devices=N_CORES)

    # all bulk inputs are host-packed partition-major [P, C, W] so each DMA
    # is one descriptor per partition (max size, minimal HWDGE desc-gen time)
    x1T = nc.dram_tensor("x1T", [P, DC, S], bf16, kind="ExternalInput").ap()
    x2gT = nc.dram_tensor("x2gT", [P, DC, k_pad], bf16, kind="ExternalInput").ap()
    pbias = nc.dram_tensor("pbias", [P, KC], f32, kind="ExternalInput").ap()
    wqT = nc.dram_tensor("wqT", [P, EC, D], bf16, kind="ExternalInput").ap()
    wkT = nc.dram_tensor("wkT", [P, EC, D], bf16, kind="ExternalInput").ap()
    wv = nc.dram_tensor("wv", [P, DC, D], bf16, kind="ExternalInput").ap()
    wo = nc.dram_tensor("wo", [P, DC, D], bf16, kind="ExternalInput").ap()
    out = nc.dram_tensor("out", [S, D], f32, kind="ExternalOutput").ap()

    with tile.TileContext(nc) as tc, ExitStack() as ctx:
        persist = ctx.enter_context(tc.tile_pool(name="persist", bufs=1))

        G2 = persist.tile([P, DC, k_pad], bf16, name="G2")    # [d1, kp]
        V = persist.tile([P, KC, D], bf16, name="V")          # [kp, e]
        x1sb = persist.tile([P, DC, S], bf16, name="x1sb")    # [d1, s]
        ones_bf = persist.tile([P, 1], bf16, name="ones_bf")
        ones_f1 = persist.tile([1, 1], f32, name="ones_f1")
        padbias = persist.tile([P, KC], f32, name="padbias")
        wo_bf = persist.tile([P, DC, D], bf16, name="wo_bf")

        nc.vector.memset(ones_bf, 1.0)
        nc.vector.memset(ones_f1, 1.0)
        # queue q0 (SWDGE, ~120GB/s, separate from the two HWDGE rings):
        # pbias, x2g (needed ~t+38us), wv (~t+70), wo (~t+120)
        nc.gpsimd.dma_start(padbias, pbias)

        # warmup: a little PE streaming while the first weight chunks land,
        # so the HAM clock-gate starts warming before real chains begin.
        with tc.tile_pool(name="warm", bufs=1) as warmp, \
                tc.tile_pool(name="warmps", bufs=1, space="PSUM") as warmps:
            wsb = warmp.tile([P, 512], bf16, name="wsb")
            nc.vector.memset(wsb, 0.0)
            wps = warmps.tile([1, 512], f32, name="wps")
            NWARM = 6
            for i in range(NWARM):
                nc.tensor.matmul(wps, lhsT=ones_bf, rhs=wsb,
                                 start=(i == 0), stop=(i == NWARM - 1))

        # ================= phase 1: W2T, G2, V =================
        with ExitStack() as pctx:
            wpool = pctx.enter_context(tc.tile_pool(name="wpool", bufs=1))
            xpool = pctx.enter_context(tc.tile_pool(name="xpool", bufs=1))

            wqT_bf = wpool.tile([P, EC, D], bf16, name="wqT_bf")  # [e, d1]
            wkT_bf = wpool.tile([P, EC, D], bf16, name="wkT_bf")  # [e, d2]
            W2T_bf = wpool.tile([P, DC, D], bf16, name="W2T_bf")  # [d2, d1]
            wv_bf = wpool.tile([P, DC, D], bf16, name="wv_bf")    # [d, e]
            x2sb = xpool.tile([P, DC, k_pad], bf16, name="x2sb")  # [d, kp]

            # Aggregate HBM->SBUF bandwidth (~320-350GB/s) is the binding
            # constraint at kernel start, so issue bytes in strict need
            # order: weights first (per-ec chunks so the first W2T chain
            # trickle-starts at ~2.5us), then x2g, wv, x1T, wo -- split
            # across both HWDGE rings.  SWDGE only carries the tiny bias.
            H = DC // 2
            for ec in range(EC):
                nc.sync.dma_start(wkT_bf[:, ec, :], wkT[:, ec, :])
                nc.scalar.dma_start(wqT_bf[:, ec, :], wqT[:, ec, :])
            nc.sync.dma_start(x2sb[:, 0:H, :], x2gT[:, 0:H, :])
            nc.scalar.dma_start(x2sb[:, H:DC, :], x2gT[:, H:DC, :])
            nc.sync.dma_start(wv_bf[:, 0:H, :], wv[:, 0:H, :])
            nc.scalar.dma_start(wv_bf[:, H:DC, :], wv[:, H:DC, :])
            nc.sync.dma_start(x1sb[:, 0:H, :], x1T[:, 0:H, :])
            nc.scalar.dma_start(x1sb[:, H:DC, :], x1T[:, H:DC, :])
            nc.sync.dma_start(wo_bf[:, 0:H, :], wo[:, 0:H, :])
            nc.scalar.dma_start(wo_bf[:, H:DC, :], wo[:, H:DC, :])

            # W2T[d2, d1] = (wk @ wq^T)^T scaled; chains over e.
            # Group A (d2c 0..3): ec-major across all 8 PSUM banks so each
            # arriving weight chunk pair immediately feeds 8 matmuls --
            # per-chunk compute (~1.7us) matches per-chunk DMA (~1.5us), so
            # the PE pipelines against the weight trickle with no idle.
            with tc.tile_pool(name="w2ps", bufs=1, space="PSUM") as w2ps:
                pwA = {}
                for d2c in range(4):
                    for h in range(2):
                        pwA[(d2c, h)] = w2ps.tile(
                            [P, 512], f32, name=f"pwA{d2c}{h}",
                            tag=f"pwA{d2c}{h}")
                for ec in range(EC):
                    for (d2c, h), pw in pwA.items():
                        nc.tensor.matmul(
                            pw, lhsT=wkT_bf[:, ec, d2c * P:(d2c + 1) * P],
                            rhs=wqT_bf[:, ec, h * 512:(h + 1) * 512],
                            start=(ec == 0), stop=(ec == EC - 1))
                for (d2c, h), pw in pwA.items():
                    nc.scalar.activation(
                        out=W2T_bf[:, d2c, h * 512:(h + 1) * 512], in_=pw,
                        func=AF.Copy, scale=QSCALE)

            ppsum = pctx.enter_context(tc.tile_pool(name="ppsum", bufs=4, space="PSUM"))
            # group B: remaining chains, weights resident by now
            for d2c in range(4, DC):
                for h in range(2):
                    pw = ppsum.tile([P, 512], f32, name="pw", tag="pp")
                    for ec in range(EC):
                        nc.tensor.matmul(
                            pw, lhsT=wkT_bf[:, ec, d2c * P:(d2c + 1) * P],
                            rhs=wqT_bf[:, ec, h * 512:(h + 1) * 512],
                            start=(ec == 0), stop=(ec == EC - 1))
                    nc.scalar.activation(
                        out=W2T_bf[:, d2c, h * 512:(h + 1) * 512], in_=pw,
                        func=AF.Copy, scale=QSCALE)

            # G2[d1, kp] = W2 @ x2g^T; chains over d2
            for d1c in range(DC):
                for (k0, kw) in _ksplits(k_pad):
                    pg = ppsum.tile([P, 512], f32, name="pg", tag="pp")
                    for d2c in range(DC):
                        nc.tensor.matmul(
                            pg[:, :kw],
                            lhsT=W2T_bf[:, d2c, d1c * P:(d1c + 1) * P],
                            rhs=x2sb[:, d2c, k0:k0 + kw],
                            start=(d2c == 0), stop=(d2c == DC - 1))
                    nc.scalar.activation(
                        out=G2[:, d1c, k0:k0 + kw], in_=pg[:, :kw],
                        func=AF.Copy)

            # v projection: V[kp, e]; chains over d, N = 512
            for kc in range(KC):
                for fh in range(2):
                    pv = ppsum.tile([P, 512], f32, name="pv", tag="pp")
                    for dc in range(DC):
                        nc.tensor.matmul(
                            pv, lhsT=x2sb[:, dc, kc * P:(kc + 1) * P],
                            rhs=wv_bf[:, dc, fh * 512:(fh + 1) * 512],
                            start=(dc == 0), stop=(dc == DC - 1))
                    nc.scalar.activation(
                        out=V[:, kc, fh * 512:(fh + 1) * 512], in_=pv,
                        func=AF.Copy)

        # ================= phase 2: attention =================
        epool = ctx.enter_context(tc.tile_pool(name="epool", bufs=2))
        zpool = ctx.enter_context(tc.tile_pool(name="zpool", bufs=2))
        opool = ctx.enter_context(tc.tile_pool(name="opool", bufs=6))
        rpool = ctx.enter_context(tc.tile_pool(name="rpool", bufs=2))
        dsbp = ctx.enter_context(tc.tile_pool(name="dsbp", bufs=2))
        spsum = ctx.enter_context(tc.tile_pool(name="spsum", bufs=2, space="PSUM"))
        ypsum = ctx.enter_context(tc.tile_pool(name="ypsum", bufs=2, space="PSUM"))
        dpsum = ctx.enter_context(tc.tile_pool(name="dpsum", bufs=1, space="PSUM"))
        tpsum = ctx.enter_context(tc.tile_pool(name="tpsum", bufs=1, space="PSUM"))
        opsum = ctx.enter_context(tc.tile_pool(name="opsum", bufs=2, space="PSUM"))

        for qt in range(NQT):
            q0 = qt * Q_TILE
            expt = epool.tile([P, KC, Q_TILE], bf16, name="expt", tag="expt")
            pdrow = dpsum.tile([1, Q_TILE], f32, name="pdrow", tag="pdrow")

            def emit_denom(kc):
                # ones-column matmul: accumulates the sum over this kp-chunk
                # of exp into the [1, Q] denominator row.  M=1 weights load
                # in ~1 cycle, so these interleave between chains ~free.
                nc.tensor.matmul(
                    pdrow, lhsT=ones_bf, rhs=expt[:, kc, :],
                    start=(kc == 0), stop=(kc == KC - 1))

            for kc in range(KC):
                ps = spsum.tile([P, Q_TILE], f32, name="ps", tag="ps")
                for dc in range(DC):
                    nc.tensor.matmul(
                        ps, lhsT=G2[:, dc, kc * P:(kc + 1) * P],
                        rhs=x1sb[:, dc, q0:q0 + Q_TILE],
                        start=(dc == 0), stop=(dc == DC - 1))
                nc.scalar.activation(
                    out=expt[:, kc, :], in_=ps, func=AF.Exp,
                    bias=padbias[:, kc:kc + 1], scale=1.0)
                if kc >= 1:
                    emit_denom(kc - 1)

            # NOTE: matmul start=True clears has_written bits for the WHOLE
            # PSUM bank, so accumulation chains must not interleave within a
            # bank.  pdrow has its own bank; only its kc==0 matmul uses
            # start=True.
            zt = zpool.tile([P, EC, Q_TILE], bf16, name="zt", tag="zt")
            pdsb = dsbp.tile([1, Q_TILE], f32, name="pdsb", tag="pdsb")
            tp = tpsum.tile([P, QS], f32, name="tp", tag="tp")
            recip = rpool.tile([P, QS], f32, name="recip", tag="recip")
            for ec in range(EC):
                py = ypsum.tile([P, Q_TILE], f32, name="py", tag="py")
                for kc in range(KC):
                    nc.tensor.matmul(
                        py, lhsT=V[:, kc, ec * P:(ec + 1) * P],
                        rhs=expt[:, kc, :],
                        start=(kc == 0), stop=(kc == KC - 1))
                nc.scalar.activation(out=zt[:, ec, :], in_=py, func=AF.Relu)
                if ec == 0:
                    emit_denom(KC - 1)
                    nc.scalar.activation(out=pdsb, in_=pdrow, func=AF.Copy)
                if ec == 1:
                    # transpose the denom row into per-partition layout:
                    # tp[p, qs] = pdsb[0, qs*128 + p] via K=1 matmuls
                    for qs in range(QS):
                        nc.tensor.matmul(
                            tp[:, qs:qs + 1],
                            lhsT=pdsb[0:1, qs * P:(qs + 1) * P],
                            rhs=ones_f1, start=(qs == 0), stop=(qs == QS - 1))
                if ec == 2:
                    nc.vector.reciprocal(recip, tp)

            for qs in range(QS):
                for fh in range(2):
                    po = opsum.tile([P, 512], f32, name="po", tag="po")
                    for ec in range(EC):
                        nc.tensor.matmul(
                            po, lhsT=zt[:, ec, qs * P:(qs + 1) * P],
                            rhs=wo_bf[:, ec, fh * 512:(fh + 1) * 512],
                            start=(ec == 0), stop=(ec == EC - 1))
                    osb = opool.tile([P, 512], f32, name="osb", tag="osb")
                    nc.scalar.activation(
                        out=osb, in_=po, func=AF.Copy,
                        scale=recip[:, qs:qs + 1])
                    orows = out[q0 + qs * P: q0 + (qs + 1) * P, :]
                    if qt == NQT - 1 and qs == QS - 1:
                        # final stores: small pieces across both rings to
                        # shrink the post-compute drain tail
                        for i in range(4):
                            eng = nc.sync if i % 2 == 0 else nc.scalar
                            c0 = fh * 512 + i * 128
                            eng.dma_start(
                                orows[:, c0:c0 + 128],
                                osb[:, i * 128:(i + 1) * 128])
                    else:
                        eng = nc.sync if (qs * 2 + fh) % 2 == 0 else nc.scalar
                        eng.dma_start(
                            orows[:, fh * 512:(fh + 1) * 512], osb)

    nc.compile()
    return nc


def _prep_inputs(x1, x2, maskSeq, wq, wk, wv, wo):
    """Host-side shard prep: bf16 cast, transpose, masked-key gather."""
    import ml_dtypes
    bf = ml_dtypes.bfloat16

    mask = np.asarray(maskSeq, dtype=np.int32)
    idxs = [np.flatnonzero(mask[b, 0] == 0) for b in range(B)]
    maxc = max(len(i) for i in idxs)
    k_pad = max(-(-maxc // P) * P, 2 * P)
    KC = k_pad // P

    def pack(a):
        # [C*128, W] -> [128, C, W]: partition-major so each partition's DMA
        # data is one contiguous run
        c = a.shape[0] // P
        return np.ascontiguousarray(a.reshape(c, P, -1).transpose(1, 0, 2))

    x1b = np.asarray(x1, dtype=np.float32).astype(bf)
    x2b = np.asarray(x2, dtype=np.float32).astype(bf)
    x2g = np.zeros((B, k_pad, D), dtype=bf)
    bias = np.empty((B, P, KC), dtype=np.float32)
    for b, idx in enumerate(idxs):
        n = len(idx)
        x2g[b, :n] = x2b[b][idx]
        bv = np.zeros(k_pad, dtype=np.float32)
        bv[n:] = -1.0e9
        bias[b] = np.ascontiguousarray(bv.reshape(KC, P).T)
    x1Tp = [pack(x1b[b].T) for b in range(B)]
    x2gTp = [pack(np.ascontiguousarray(x2g[b].T)) for b in range(B)]

    wqT = pack(np.asarray(wq, dtype=np.float32).astype(bf).T)
    wkT = pack(np.asarray(wk, dtype=np.float32).astype(bf).T)
    wv = pack(np.asarray(wv, dtype=np.float32).astype(bf))
    wo = pack(np.asarray(wo, dtype=np.float32).astype(bf))

    in_maps = [
        {"x1T": x1Tp[c], "x2gT": x2gTp[c], "pbias": bias[c],
         "wqT": wqT, "wkT": wkT, "wv": wv, "wo": wo}
        for c in range(N_CORES)
    ]
    return k_pad, in_maps


def kernel(x1, x2, maskSeq, wq, wk, wv, wo, **_unused):
    from concourse.bass_utils import run_bass_kernel_spmd

    k_pad, in_maps = _prep_inputs(x1, x2, maskSeq, wq, wk, wv, wo)
    if k_pad not in _cached_nc:
        _cached_nc[k_pad] = _build(k_pad)
    nc = _cached_nc[k_pad]

    res = run_bass_kernel_spmd(nc, in_maps, core_ids=list(range(N_CORES)))
    return np.stack([res.results[c]["out"] for c in range(N_CORES)], axis=0)
